# revision 1
# baseline (speedup 1.0000x reference)
"""Self-contained Trainium2 Bass kernel: causal self-attention, 8-core SPMD.

nn_CausalSelfAttention: B=4, T=2048, C=1024, n_head=16 (fp32 reference).

Sharding (hardcoded): core c -> batch b = c//2, head-group g = c%2
(8 of 16 heads = 512 features). Data parallel over B, tensor parallel
over heads. Each core computes a partial output [T, C] = y_g @ Wp_g^T;
the host sums the two partials per batch and adds bp (the tensor-parallel
all-reduce done at unshard time).

Device kernel (per core), v2 — restructured from the v1 baseline for
engine overlap and less wasted work:
  - softmax denominators broadcast across partitions via a K=2 PE
    outer-product matmul into PSUM (v1 used a DRAM DMA bounce: 8MB of
    broadcast DMA traffic + ~2-4us latency on the critical path of
    every pair); reciprocal via the 1-op DVE approx (v1: 53us of
    iterative InstReciprocal)
  - causal diagonal tiles compute only the valid upper-trapezoid
    column range (saves PE columns, exp elements, and shrinks the
    mask multiply to one resident 128x128 triangle)
  - all PSUM->SBUF evictions moved to GpSimd; ScalarE does exp only
  - x preloaded whole; QKV projections of block jb+1 and the output
    projection of block jb-1 run as paced fillers inside the
    attention stream of block jb so the PE never starves on the
    softmax latency chain
~5e-4 relative error vs the fp32 reference.
"""

import sys
from collections import deque

for _p in ("/opt/trn_rl_repo",):
    if _p not in sys.path:
        sys.path.insert(0, _p)

import numpy as np

import concourse.bacc as bacc
import concourse.bass as bass
import concourse.tile as tile
from concourse import mybir

F32 = mybir.dt.float32
F16 = mybir.dt.float16

T = 2048
C = 1024
O = 512          # per-core output features (8 heads x 64)
HD = 64
NJB = 4          # tq blocks of 512
NCC = 8          # c chunks of 128
NOC = 4          # o chunks of 128
SCALE = 1.0 / 8.0  # 1/sqrt(64), folded into Wq/bq host-side


def build(debug=False):
    np_dt = np.float16
    sb_dt = F16

    nc = bacc.Bacc("TRN2", target_bir_lowering=False, debug=False)

    # all large inputs come as SBUF images ([128 partitions, ...]) so each
    # DMA reads 128 contiguous multi-KB rows instead of 1024 strided 1KB rows
    xt_d = nc.dram_tensor("xt", [NJB, 128, NCC, 512], sb_dt, kind="ExternalInput").ap()
    wqt_d = nc.dram_tensor("wqt", [128, NCC, O], sb_dt, kind="ExternalInput").ap()
    wkt_d = nc.dram_tensor("wkt", [128, NCC, O], sb_dt, kind="ExternalInput").ap()
    wvt_d = nc.dram_tensor("wvt", [128, NCC, O], sb_dt, kind="ExternalInput").ap()
    wpt_d = nc.dram_tensor("wpt", [128, NOC, C], sb_dt, kind="ExternalInput").ap()
    bq_d = nc.dram_tensor("bq", [128, NOC], F32, kind="ExternalInput").ap()
    bk_d = nc.dram_tensor("bk", [128, NOC], F32, kind="ExternalInput").ap()
    bvb_d = nc.dram_tensor("bvb", [128, O], F32, kind="ExternalInput").ap()
    tri_d = nc.dram_tensor("tri", [128, 128], sb_dt, kind="ExternalInput").ap()
    out_d = nc.dram_tensor("out", [T, C], F32, kind="ExternalOutput").ap()
    if debug:
        dben_d = nc.dram_tensor("dben", [NJB, 4, 2, 512], F32, kind="ExternalOutput").ap()
        drec_d = nc.dram_tensor("drec", [NJB, 4, 2, 512], F32, kind="ExternalOutput").ap()
        dbc_d = nc.dram_tensor("dbc", [NJB, 4, 2, 512], F32, kind="ExternalOutput").ap()

    with tile.TileContext(nc) as tc:
        with (
            tc.tile_pool(name="const", bufs=1) as const,
            tc.tile_pool(name="qt_pool", bufs=2) as qt_pool,
            tc.tile_pool(name="att_pool", bufs=4) as att_pool,
            tc.tile_pool(name="yt_pool", bufs=2) as yt_pool,
            tc.tile_pool(name="den_pool", bufs=2) as den_pool,
            tc.tile_pool(name="misc", bufs=2) as misc,
            tc.tile_pool(name="ost_pool", bufs=3) as ost_pool,
            tc.tile_pool(name="pst", bufs=2, space="PSUM") as pst,
            tc.tile_pool(name="pa", bufs=2, space="PSUM") as pa,
            tc.tile_pool(name="pav", bufs=2, space="PSUM") as pav,
        ):
            # ---- constants / weights (resident) ----
            wq_sb = const.tile([128, NCC, O], sb_dt, name="wq_sb")
            wk_sb = const.tile([128, NCC, O], sb_dt, name="wk_sb")
            wv_sb = const.tile([128, NCC, O], sb_dt, name="wv_sb")
            wp_sb = const.tile([128, NOC, C], sb_dt, name="wp_sb")
            xt_sb = const.tile([128, NJB, NCC, 512], sb_dt, name="xt_sb")
            # q-proj inputs first on their queues so stage A(0) starts ASAP
            nc.sync.dma_start(out=wq_sb, in_=wqt_d)
            nc.scalar.dma_start(out=xt_sb[:, 0], in_=xt_d[0])
            nc.gpsimd.dma_start(out=wk_sb, in_=wkt_d)
            nc.sync.dma_start(out=wv_sb, in_=wvt_d)
            nc.scalar.dma_start(out=xt_sb[:, 1], in_=xt_d[1])
            nc.gpsimd.dma_start(out=wp_sb, in_=wpt_d)
            nc.sync.dma_start(out=xt_sb[:, 2], in_=xt_d[2])
            nc.scalar.dma_start(out=xt_sb[:, 3], in_=xt_d[3])

            bq_sb = const.tile([128, NOC], F32, name="bq_sb")
            bk_sb = const.tile([128, NOC], F32, name="bk_sb")
            bvb_sb = const.tile([128, O], F32, name="bvb_sb")
            tri_sb = const.tile([128, 128], sb_dt, name="tri_sb")
            nc.gpsimd.dma_start(out=bq_sb, in_=bq_d)
            nc.gpsimd.dma_start(out=bk_sb, in_=bk_d)
            nc.gpsimd.dma_start(out=bvb_sb, in_=bvb_d)
            nc.gpsimd.dma_start(out=tri_sb, in_=tri_d)

            # denominator staging: two const tiles used alternately by the
            # pair finalizes (rows 1-31 stay at the 1.0 memset so the packed
            # iterative reciprocal never sees garbage)
            den_tiles = []
            for dd in range(2):
                dt_ = const.tile([33, 1024], F32, name=f"den{dd}")
                nc.vector.memset(dt_[:, 0:512], 1.0)
                den_tiles.append(dt_)

            # ones row for the K=1 outer-product denominator broadcast,
            # zeros row for the jb=0 group-closing matmul
            ones_sb = const.tile([1, 128], sb_dt, name="ones_sb")
            nc.vector.memset(ones_sb, 1.0)
            zero16 = const.tile([1, 512], sb_dt, name="zero16")
            nc.vector.memset(zero16, 0.0)

            # persistent K^T and V per (chunk, block). V carries a ones
            # column per head ([v | 1]) so the AV matmul (M=65) also
            # accumulates the softmax denominator in its row 64.
            kt_t = {}
            v_t = {}
            for jbx in range(NJB):
                for oc in range(NOC):
                    kt_t[oc, jbx] = const.tile(
                        [128, 512], sb_dt, name=f"kt{oc}_{jbx}"
                    )
                v_t[jbx] = const.tile([128, 4, 8, 65], sb_dt, name=f"v_{jbx}")
                nc.vector.memset(v_t[jbx][:, :, :, 64:65], 1.0)

            qt_tiles = {}
            yt_tiles = {}

            # ---- stage A: QKV projection groups for t-block jb ----
            def qk_group(jb, oc, mat):
                def emit():
                    if mat == 0 and oc == 0:
                        qt_tiles[jb] = qt_pool.tile(
                            [128, NOC, 512], sb_dt, tag="qt", name="qt"
                        )
                    w_sb = wq_sb if mat == 0 else wk_sb
                    ps = pa.tile([128, 512], F32, tag="apsum", name=f"qk{jb}{oc}{mat}")
                    for cc in range(NCC):
                        nc.tensor.matmul(
                            ps,
                            lhsT=w_sb[:, cc, 128 * oc : 128 * (oc + 1)],
                            rhs=xt_sb[:, jb, cc, :],
                            start=(cc == 0),
                            stop=(cc == NCC - 1),
                        )
                    if mat == 0:
                        nc.scalar.activation(
                            qt_tiles[jb][:, oc, :], ps,
                            mybir.ActivationFunctionType.Identity,
                            bias=bq_sb[:, oc : oc + 1], scale=1.0,
                        )
                    else:
                        nc.scalar.activation(
                            kt_t[oc, jb], ps,
                            mybir.ActivationFunctionType.Identity,
                            bias=bk_sb[:, oc : oc + 1], scale=1.0,
                        )
                return emit

            def v_group(jb, tt):
                def emit():
                    ps = pa.tile([128, 512], F32, tag="apsum", name=f"v{jb}{tt}")
                    for cc in range(NCC):
                        nc.tensor.matmul(
                            ps,
                            lhsT=xt_sb[:, jb, cc, 128 * tt : 128 * (tt + 1)],
                            rhs=wv_sb[:, cc, :],
                            start=(cc == 0),
                            stop=(cc == NCC - 1),
                        )
                    nc.vector.scalar_tensor_tensor(
                        v_t[jb][:, tt, :, 0:64],
                        ps.rearrange("p (h d) -> p h d", h=8),
                        0.0,
                        bvb_sb.rearrange("p (h d) -> p h d", h=8),
                        op0=mybir.AluOpType.add,
                        op1=mybir.AluOpType.add,
                    )
                return emit

            def a_groups(jb):
                gs = [qk_group(jb, 0, 0), qk_group(jb, 0, 1)]
                gs += [v_group(jb, tt) for tt in range(4)]
                for oc in range(1, 4):
                    gs += [qk_group(jb, oc, 0), qk_group(jb, oc, 1)]
                return gs

            # ---- stage C: output projection group for t-block jb ----
            def c_group(jb, cb, tt):
                def emit():
                    op = pav.tile([128, 512], F32, tag="av", name=f"op{jb}{cb}{tt}")
                    yt_c = yt_tiles[jb]
                    for oc in range(NOC):
                        nc.tensor.matmul(
                            op,
                            lhsT=yt_c[oc][:, 128 * tt : 128 * (tt + 1)],
                            rhs=wp_sb[:, oc, 512 * cb : 512 * (cb + 1)],
                            start=(oc == 0),
                            stop=(oc == NOC - 1),
                        )
                    ost = ost_pool.tile([128, 512], F32, tag="ost", name="ost")
                    nc.vector.tensor_copy(ost, op)
                    nc.sync.dma_start(
                        out=out_d[
                            512 * jb + 128 * tt : 512 * jb + 128 * (tt + 1),
                            512 * cb : 512 * (cb + 1),
                        ],
                        in_=ost,
                    )
                return emit

            def c_groups(jb):
                return [c_group(jb, cb, tt) for cb in range(2) for tt in range(4)]

            # ---- stage B tile: QK^T scores -> exp -> (mask) -> AV ----
            # split into qk-part and av-part so the pair finalize (which
            # frees the "av" PSUM slots) can sit between them at pair start
            def tile_qk_part(jb, p, tsb):
                diag = tsb >= 4 * jb
                r = tsb - 4 * jb
                col0 = 128 * r if diag else 0
                st = pst.tile([128, 1024], F32, tag="st", name="st")
                for r2 in range(2):
                    nc.tensor.matmul(
                        st[:, 512 * r2 + col0 : 512 * (r2 + 1)],
                        lhsT=kt_t[p, tsb // 4][
                            64 * r2 : 64 * (r2 + 1),
                            128 * (tsb % 4) : 128 * (tsb % 4 + 1),
                        ],
                        rhs=qt_tiles[jb][64 * r2 : 64 * (r2 + 1), p, col0:512],
                        tile_position=(64 * r2, 0),
                        start=True,
                        stop=True,
                    )
                att = att_pool.tile([128, 1024], sb_dt, tag="att", name="att")
                if col0:
                    st_v = st.rearrange("p (h q) -> p h q", h=2)[:, :, col0:512]
                    att_v = att.rearrange("p (h q) -> p h q", h=2)[:, :, col0:512]
                    nc.scalar.activation(
                        att_v, st_v, mybir.ActivationFunctionType.Exp
                    )
                else:
                    nc.scalar.activation(
                        att, st, mybir.ActivationFunctionType.Exp
                    )
                if diag:
                    for r2 in range(2):
                        sl = slice(512 * r2 + col0, 512 * r2 + col0 + 128)
                        nc.gpsimd.tensor_mul(att[:, sl], att[:, sl], tri_sb)
                return att, col0

            def tile_av_part(jb, p, tsb, att, col0, avpa, avpb, first, last):
                for r2, avp in ((0, avpa), (1, avpb)):
                    h = 2 * p + r2
                    nc.tensor.matmul(
                        avp[0:65, col0:512],
                        lhsT=v_t[tsb // 4][:, tsb % 4, h, :],
                        rhs=att[:, 512 * r2 + col0 : 512 * (r2 + 1)],
                        start=first,
                        stop=last,
                    )

            # ---- pair finalize, two phases ----
            # phase 1 (DVE only, at next pair's first tile): stage y and
            # denominators to SBUF (frees the AV banks), start the exact
            # iterative reciprocal (~3.3us).
            # phase 2 (a few tiles later so the reciprocal never blocks the
            # in-order PE queue): f16 conversions, PE outer-product
            # broadcast of 1/den, and the normalize multiplies.
            def finalize1(jb, p, avpa, avpb):
                if jb == 0:
                    # all tiles of a jb=0 pair are diagonal-restricted, so no
                    # full-width AV ever closes the accumulation group; close
                    # it with a zero-rhs matmul (adds nothing, sets stop)
                    for avp in (avpa, avpb):
                        nc.tensor.matmul(
                            avp[0:65, :],
                            lhsT=ones_sb[:, 0:65],
                            rhs=zero16,
                            start=False,
                            stop=True,
                        )
                yra = misc.tile([65, 512], F32, tag="yra", name="yra")
                yrb = misc.tile([65, 512], F32, tag="yrb", name="yrb")
                nc.vector.tensor_copy(yra, avpa[0:65, :])
                nc.vector.tensor_copy(yrb, avpb[0:65, :])
                den2 = den_tiles[(4 * jb + p) % 2]
                nc.vector.tensor_copy(den2[0:1, 0:512], yra[64:65, :])
                nc.vector.tensor_copy(den2[32:33, 0:512], yrb[64:65, :])
                nc.vector.reciprocal(den2[0:33, 512:1024], den2[0:33, 0:512])
                return (jb, p, yra, yrb, den2)

            def finalize2(state):
                jb, p, yra, yrb, den2 = state
                den16a = den_pool.tile([1, 512], sb_dt, tag="den16a", name="den16a")
                den16b = den_pool.tile([1, 512], sb_dt, tag="den16b", name="den16b")
                nc.vector.tensor_copy(den16a, den2[0:1, 512:1024])
                nc.vector.tensor_copy(den16b, den2[32:33, 512:1024])
                bca = pa.tile([64, 512], F32, tag="apsum", name=f"bca{jb}{p}")
                bcb = pa.tile([64, 512], F32, tag="apsum", name=f"bcb{jb}{p}")
                nc.tensor.matmul(
                    bca, lhsT=ones_sb[:, 0:64], rhs=den16a,
                    start=True, stop=True,
                )
                nc.tensor.matmul(
                    bcb, lhsT=ones_sb[:, 0:64], rhs=den16b,
                    start=True, stop=True,
                )
                if p == 0:
                    yt_tiles[jb] = [
                        yt_pool.tile(
                            [128, 512], sb_dt, tag=f"yt{_o}", name=f"yt{_o}"
                        )
                        for _o in range(NOC)
                    ]
                yt = yt_tiles[jb][p]
                nc.vector.tensor_mul(yt[0:64, :], yra[0:64, :], bca)
                nc.vector.tensor_mul(yt[64:128, :], yrb[0:64, :], bcb)

            # ---- schedule ----
            # stage A(0) as an upfront burst, then walk the attention tiles
            # of B(jb) with A(jb+1) paced mid-pair and C(jb-1) pumped after
            # same-jb finalizes (never mid-pair: C shares the "av" PSUM ring
            # with the open accumulators -> would deadlock).
            for g in a_groups(0):
                g()

            a_q = deque()
            c_q = deque()
            pend = None   # (jb, p, avpa, avpb) awaiting finalize1
            fin2 = None   # finalize1 state awaiting finalize2
            fin2_age = 0

            for jb in range(NJB):
                a_q.extend(a_groups(jb + 1) if jb + 1 < NJB else [])
                c_q.extend(c_groups(jb - 1) if jb >= 1 else [])
                n_ts = 4 * jb + 4
                a_stride = {0: 1, 1: 2, 2: 4, 3: 64}[jb]
                # diagonal tiles first: the r=0 diag tile is full-width
                # (starts the group), later full-width off-diag tiles close
                # it with stop=True; short diag exps also warm the Scalar
                # pipeline right at pair start
                if jb == 0:
                    ts_order = list(range(4))
                else:
                    ts_order = (
                        [0] + list(range(4 * jb, n_ts)) + list(range(1, 4 * jb))
                    )
                for p in range(4):
                    avpa = avpb = None
                    avq = deque()  # tiles awaiting their AV emission
                    def flush_av(jb=jb, p=p, n_ts=n_ts):
                        idx, tsb, att, col0 = avq.popleft()
                        tile_av_part(
                            jb, p, tsb, att, col0, avpa, avpb,
                            first=(idx == 0),
                            last=(idx == n_ts - 1 and jb > 0),
                        )
                    for idx, tsb in enumerate(ts_order):
                        att, col0 = tile_qk_part(jb, p, tsb)
                        avq.append((idx, tsb, att, col0))
                        if idx == 0:
                            if pend is not None:
                                if a_q:
                                    a_q.popleft()()
                                fin2 = finalize1(*pend)
                                fin2_age = 0
                                # C groups are safe only here: finalize1
                                # released the av banks (yra/yrb copies are
                                # the last avp readers) and no accumulation
                                # is open. Skip the first boundary of each
                                # jb: the previous jb's last finalize2 (and
                                # its yt writes) hasn't been emitted yet.
                                if pend[0] == jb:
                                    for _ in range(3):
                                        if c_q:
                                            c_q.popleft()()
                                pend = None
                            # allocate after the boundary work so the "av"
                            # ring order matches first-use order
                            avpa = pav.tile(
                                [128, 512], F32, tag="av", name=f"avpa{p}"
                            )
                            avpb = pav.tile(
                                [128, 512], F32, tag="av", name=f"avpb{p}"
                            )
                        # QK runs 2 tiles ahead of AV so the PE always has
                        # independent work while the av banks free up at
                        # pair boundaries (gaps >100ns also drop the PE
                        # clock out of its top p-state)
                        if len(avq) > 2:
                            flush_av()
                        if fin2 is not None:
                            fin2_age += 1
                            if fin2_age >= 6 or idx == n_ts - 1:
                                finalize2(fin2)
                                fin2 = None
                        if idx % a_stride == a_stride - 1 and a_q:
                            a_q.popleft()()
                    while avq:
                        flush_av()
                    pend = (jb, p, avpa, avpb)

            # drain: finalize last pair, then the final output projection
            fin2 = finalize1(*pend)
            finalize2(fin2)
            while c_q:
                c_q.popleft()()
            for g in c_groups(3):
                g()

    nc.finalize()
    return nc, {"np_dt": np_dt}


def shard_inputs(inputs, np_dt):
    """Full inputs -> list of 8 per-core input dicts."""
    q = np.asarray(inputs["query"], np.float32)
    Wq = np.asarray(inputs["Wq"], np.float32) * np.float32(SCALE)
    Wk = np.asarray(inputs["Wk"], np.float32)
    Wv = np.asarray(inputs["Wv"], np.float32)
    Wp = np.asarray(inputs["Wp"], np.float32)
    bq = np.asarray(inputs["bq"], np.float32)
    bk = np.asarray(inputs["bk"], np.float32)
    bv = np.asarray(inputs["bv"], np.float32)
    tri = (np.arange(128)[None, :] >= np.arange(128)[:, None]).astype(np_dt)
    in_maps = []
    for core in range(8):
        b, g = core // 2, core % 2
        sl = slice(O * g, O * (g + 1))
        in_maps.append({
            "xt": np.ascontiguousarray(
                q[b].T.reshape(8, 128, NJB, 512).transpose(2, 1, 0, 3)
            ).astype(np_dt),
            "wqt": np.ascontiguousarray(
                Wq[sl, :].T.reshape(8, 128, O).transpose(1, 0, 2)
            ).astype(np_dt),
            "wkt": np.ascontiguousarray(
                Wk[sl, :].T.reshape(8, 128, O).transpose(1, 0, 2)
            ).astype(np_dt),
            "wvt": np.ascontiguousarray(
                Wv[sl, :].T.reshape(8, 128, O).transpose(1, 0, 2)
            ).astype(np_dt),
            "wpt": np.ascontiguousarray(
                Wp[:, sl].T.reshape(NOC, 128, C).transpose(1, 0, 2)
            ).astype(np_dt),
            "bq": np.ascontiguousarray(bq[sl].reshape(NOC, 128).T) * np.float32(SCALE),
            "bk": np.ascontiguousarray(bk[sl].reshape(NOC, 128).T),
            "bvb": np.broadcast_to(bv[sl], (128, O)).copy(),
            "tri": tri,
        })
    return in_maps


def unshard(results, bp):
    out = np.empty((4, T, C), np.float32)
    for b in range(4):
        out[b] = results[2 * b]["out"] + results[2 * b + 1]["out"] + np.asarray(
            bp, np.float32
        )
    return out


_CACHE = {}


def _get_nc(mode="f16"):
    if mode not in _CACHE:
        _CACHE[mode] = build()
    return _CACHE[mode]


def kernel(**inputs):
    """Full unsharded inputs -> full [4, 2048, 1024] fp32 output."""
    from concourse import bass_utils

    nc, meta = _get_nc("f16")
    in_maps = shard_inputs(inputs, meta["np_dt"])
    res = bass_utils.run_bass_kernel_spmd(nc, in_maps, list(range(8)))
    return unshard(res.results, inputs["bp"])



# revision 9
# speedup vs baseline: 1.1427x; 1.1427x over previous
"""Self-contained Trainium2 Bass kernel: causal self-attention, 8-core SPMD.

nn_CausalSelfAttention: B=4, T=2048, C=1024, n_head=16 (fp32 reference).

Sharding (hardcoded): core c -> batch b = c//2, head-group g = c%2
(8 of 16 heads = 512 features). Data parallel over B, tensor parallel
over heads. Each core computes a partial output [T, C] = y_g @ Wp_g^T;
the host sums the two partials per batch and adds bp (the tensor-parallel
all-reduce done at unshard time).

v3 changes over the 353us v2 baseline (trace-driven):
  - pair-finalize reciprocal switched to the 1-op approx DVE reciprocal
    (3.34us InstReciprocal -> ~0.7us) so the DVE in-order queue no
    longer backs up behind it at pair boundaries (the v2 trace showed
    2.4-2.8us PE gaps at every boundary, each re-throttling the PE
    clock to 1.2 GHz for ~10us via HAM)
  - q/k bias evictions moved from ScalarE (IDENTITY, 22us) to DVE
    tensor_scalar so ScalarE runs exp only; exp throughput is the
    secondary bottleneck in the late (large-jb) phases
  - output-projection groups moved off the "av" PSUM ring onto the
    "pa" ring so they never wait on pair-finalize copies
  - den broadcast packed into one [128,512] PSUM tile via two
    column-tiled concurrent matmuls (tile_position (0,0)/(0,64))
  - diag tiles processed LAST within each pair so stage-A groups of
    block jb+1 can spill into B(jb+1) itself; filler emission is
    driven by a static PE-vs-ACT cost model instead of fixed strides
  - initial DMAs split across 4 queue engines (v2 spent 19us before
    the first matmul); fp16 partial outputs (halves the output DMA)
~6e-4 relative error vs the fp32 reference.
"""

import sys
from collections import deque

for _p in ("/opt/trn_rl_repo",):
    if _p not in sys.path:
        sys.path.insert(0, _p)

import numpy as np

import concourse.bacc as bacc
import concourse.bass as bass
import concourse.tile as tile
from concourse import mybir

F32 = mybir.dt.float32
F16 = mybir.dt.float16

T = 2048
C = 1024
O = 512          # per-core output features (8 heads x 64)
HD = 64
NJB = 4          # tq blocks of 512
NCC = 8          # c chunks of 128
NOC = 4          # o chunks of 128
SCALE = 1.0 / 8.0  # 1/sqrt(64), folded into Wq/bq host-side

# cost model (ns) for the static filler scheduler
MM_NS = 216.0        # N=512 matmul, warm
EXP_OVH = 352 / 1.2  # ACT per-instruction overhead


def act_cost(col0):
    """exp cost for one tile (2 heads), diag-restricted to [col0:512]."""
    return (2 * (512 - col0) + 352) / 1.2


def pe_att_cost(col0):
    """QK (row-tiled concurrent pair) + 2 AV matmuls for one tile."""
    return (3 * (512 - col0)) / 2.4 + 24.0


def build(debug=False):
    np_dt = np.float16
    sb_dt = F16

    nc = bacc.Bacc("TRN2", target_bir_lowering=False, debug=False)

    # all large inputs come as SBUF images ([128 partitions, ...]) so each
    # DMA reads 128 contiguous multi-KB rows instead of 1024 strided 1KB rows
    xt_d = nc.dram_tensor("xt", [NJB, 128, NCC, 512], sb_dt, kind="ExternalInput").ap()
    wqt_d = nc.dram_tensor("wqt", [128, NCC, O], sb_dt, kind="ExternalInput").ap()
    wkt_d = nc.dram_tensor("wkt", [128, NCC, O], sb_dt, kind="ExternalInput").ap()
    wvt_d = nc.dram_tensor("wvt", [128, NCC, O], sb_dt, kind="ExternalInput").ap()
    wpt_d = nc.dram_tensor("wpt", [128, NOC, C], sb_dt, kind="ExternalInput").ap()
    bq_d = nc.dram_tensor("bq", [128, NOC], F32, kind="ExternalInput").ap()
    bk_d = nc.dram_tensor("bk", [128, NOC], F32, kind="ExternalInput").ap()
    bvb_d = nc.dram_tensor("bvb", [128, O], F32, kind="ExternalInput").ap()
    tri_d = nc.dram_tensor("tri", [128, 128], sb_dt, kind="ExternalInput").ap()
    out_d = nc.dram_tensor("out", [T, C], sb_dt, kind="ExternalOutput").ap()

    with tile.TileContext(nc) as tc:
        with (
            tc.tile_pool(name="const", bufs=1) as const,
            tc.tile_pool(name="qt_pool", bufs=2) as qt_pool,
            tc.tile_pool(name="att_pool", bufs=4) as att_pool,
            tc.tile_pool(name="yt_pool", bufs=16) as yt_pool,
            tc.tile_pool(name="den_pool", bufs=2) as den_pool,
            tc.tile_pool(name="misc", bufs=4) as misc,
            tc.tile_pool(name="ost_pool", bufs=3) as ost_pool,
            tc.tile_pool(name="pst", bufs=2, space="PSUM") as pst,
            tc.tile_pool(name="pa", bufs=2, space="PSUM") as pa,
            tc.tile_pool(name="pav", bufs=2, space="PSUM") as pav,
        ):
            # ---- constants / weights ----
            # small tensors first (biases/tri are needed by the first
            # evictions), then the first-group-critical big tensors split
            # across the 4 queue engines so the first matmul can start
            # ~7us in instead of 19us.
            wq_sb = const.tile([128, NCC, O], sb_dt, name="wq_sb")
            wk_sb = const.tile([128, NCC, O], sb_dt, name="wk_sb")
            wv_sb = const.tile([128, NCC, O], sb_dt, name="wv_sb")
            wp_sb = const.tile([128, NOC, C], sb_dt, name="wp_sb")
            xt_sb = const.tile([128, NJB, NCC, 512], sb_dt, name="xt_sb")
            bq_sb = const.tile([128, NOC], F32, name="bq_sb")
            bk_sb = const.tile([128, NOC], F32, name="bk_sb")
            bvb_sb = const.tile([128, O], F32, name="bvb_sb")
            tri_sb = const.tile([128, 128], sb_dt, name="tri_sb")

            qs = [nc.sync, nc.scalar, nc.gpsimd]
            nc.sync.dma_start(out=bq_sb, in_=bq_d)
            nc.scalar.dma_start(out=bk_sb, in_=bk_d)
            nc.gpsimd.dma_start(out=bvb_sb, in_=bvb_d)
            nc.sync.dma_start(out=tri_sb, in_=tri_d)
            # first q-projection group needs wq[:, :, 0:128] + xt[0];
            # interleave those chunks round-robin over the 3 queue engines
            nc.scalar.dma_start(out=wq_sb[:, :, 0:128], in_=wqt_d[:, :, 0:128])
            for cc in range(4):
                qs[cc % 3].dma_start(
                    out=xt_sb[:, 0, 2 * cc : 2 * cc + 2],
                    in_=xt_d[0, :, 2 * cc : 2 * cc + 2],
                )
            nc.gpsimd.dma_start(out=wq_sb[:, :, 128:256], in_=wqt_d[:, :, 128:256])
            nc.sync.dma_start(out=wq_sb[:, :, 256:512], in_=wqt_d[:, :, 256:512])
            for i in range(4):
                qs[i % 3].dma_start(
                    out=wk_sb[:, :, 128 * i : 128 * (i + 1)],
                    in_=wkt_d[:, :, 128 * i : 128 * (i + 1)],
                )
            nc.scalar.dma_start(out=wv_sb[:, :, 0:256], in_=wvt_d[:, :, 0:256])
            nc.gpsimd.dma_start(out=wv_sb[:, :, 256:512], in_=wvt_d[:, :, 256:512])
            nc.sync.dma_start(out=xt_sb[:, 1], in_=xt_d[1])
            nc.scalar.dma_start(out=xt_sb[:, 2], in_=xt_d[2])
            nc.gpsimd.dma_start(out=xt_sb[:, 3], in_=xt_d[3])
            nc.sync.dma_start(out=wp_sb, in_=wpt_d)

            # denominator staging: two const tiles used alternately by the
            # pair finalizes (rows 1-31 hold the 1.0 memset so the packed
            # approximate reciprocal never sees garbage; partition bases
            # must be 32-aligned, hence rows 0 and 32 for the two heads)
            den_tiles = []
            for dd in range(2):
                dt_ = const.tile([33, 1024], F32, name=f"den{dd}")
                nc.vector.memset(dt_[:, 0:512], 1.0)
                den_tiles.append(dt_)

            # ones row for the K=1 den broadcast + zero row for the jb=0
            # group-closing matmul
            ones_sb = const.tile([1, 128], sb_dt, name="ones_sb")
            nc.vector.memset(ones_sb, 1.0)
            zero16 = const.tile([1, 512], sb_dt, name="zero16")
            nc.vector.memset(zero16, 0.0)

            # persistent K^T and V per (chunk, block). V carries a ones
            # column per head ([v | 1]) so the AV matmul (M=65) also
            # accumulates the softmax denominator in its row 64.
            kt_t = {}
            v_t = {}
            for jbx in range(NJB):
                for oc in range(NOC):
                    kt_t[oc, jbx] = const.tile(
                        [128, 512], sb_dt, name=f"kt{oc}_{jbx}"
                    )
                v_t[jbx] = const.tile([128, 4, 8, 65], sb_dt, name=f"v_{jbx}")
                nc.vector.memset(v_t[jbx][:, :, :, 64:65], 1.0)

            qt_tiles = {}
            yt_tiles = {}

            # ---- stage A: QKV projection groups for t-block jb ----
            def qk_group(jb, oc, mat):
                def emit():
                    if mat == 0 and oc == 0:
                        qt_tiles[jb] = qt_pool.tile(
                            [128, NOC, 512], sb_dt, tag="qt", name="qt"
                        )
                    w_sb = wq_sb if mat == 0 else wk_sb
                    ps = pa.tile([128, 512], F32, tag="apsum", name=f"qk{jb}{oc}{mat}")
                    for cc in range(NCC):
                        nc.tensor.matmul(
                            ps,
                            lhsT=w_sb[:, cc, 128 * oc : 128 * (oc + 1)],
                            rhs=xt_sb[:, jb, cc, :],
                            start=(cc == 0),
                            stop=(cc == NCC - 1),
                        )
                    if mat == 0:
                        nc.vector.tensor_scalar_add(
                            qt_tiles[jb][:, oc, :], ps, bq_sb[:, oc : oc + 1]
                        )
                    else:
                        nc.vector.tensor_scalar_add(
                            kt_t[oc, jb], ps, bk_sb[:, oc : oc + 1]
                        )
                return emit

            def v_group(jb, tt):
                def emit():
                    ps = pa.tile([128, 512], F32, tag="apsum", name=f"v{jb}{tt}")
                    for cc in range(NCC):
                        nc.tensor.matmul(
                            ps,
                            lhsT=xt_sb[:, jb, cc, 128 * tt : 128 * (tt + 1)],
                            rhs=wv_sb[:, cc, :],
                            start=(cc == 0),
                            stop=(cc == NCC - 1),
                        )
                    nc.vector.scalar_tensor_tensor(
                        v_t[jb][:, tt, :, 0:64],
                        ps.rearrange("p (h d) -> p h d", h=8),
                        0.0,
                        bvb_sb.rearrange("p (h d) -> p h d", h=8),
                        op0=mybir.AluOpType.add,
                        op1=mybir.AluOpType.add,
                    )
                return emit

            def a_groups(jb):
                gs = [qk_group(jb, 0, 0), qk_group(jb, 0, 1)]
                gs += [v_group(jb, tt) for tt in range(4)]
                for oc in range(1, 4):
                    gs += [qk_group(jb, oc, 0), qk_group(jb, oc, 1)]
                return gs

            # ---- stage C: output projection group for t-block jb ----
            def c_group(jb, cb, tt):
                def emit():
                    op = pa.tile([128, 512], F32, tag="apsum", name=f"op{jb}{cb}{tt}")
                    yt_c = yt_tiles[jb]
                    for oc in range(NOC):
                        nc.tensor.matmul(
                            op,
                            lhsT=yt_c[oc][:, 128 * tt : 128 * (tt + 1)],
                            rhs=wp_sb[:, oc, 512 * cb : 512 * (cb + 1)],
                            start=(oc == 0),
                            stop=(oc == NOC - 1),
                        )
                    ost = ost_pool.tile([128, 512], sb_dt, tag="ost", name="ost")
                    nc.vector.tensor_copy(ost, op)
                    dq = qs[(cb * 4 + tt) % 3]
                    dq.dma_start(
                        out=out_d[
                            512 * jb + 128 * tt : 512 * jb + 128 * (tt + 1),
                            512 * cb : 512 * (cb + 1),
                        ],
                        in_=ost,
                    )
                return emit

            def c_groups(jb):
                return [c_group(jb, cb, tt) for cb in range(2) for tt in range(4)]

            # ---- stage B tile: QK^T scores -> exp -> (mask) -> AV ----
            def tile_qk_part(jb, p, tsb):
                diag = tsb >= 4 * jb
                r = tsb - 4 * jb
                col0 = 128 * r if diag else 0
                st = pst.tile([128, 1024], F32, tag="st", name="st")
                for r2 in range(2):
                    nc.tensor.matmul(
                        st[:, 512 * r2 + col0 : 512 * (r2 + 1)],
                        lhsT=kt_t[p, tsb // 4][
                            64 * r2 : 64 * (r2 + 1),
                            128 * (tsb % 4) : 128 * (tsb % 4 + 1),
                        ],
                        rhs=qt_tiles[jb][64 * r2 : 64 * (r2 + 1), p, col0:512],
                        tile_position=(64 * r2, 0),
                        start=True,
                        stop=True,
                    )
                att = att_pool.tile([128, 1024], sb_dt, tag="att", name="att")
                if col0:
                    st_v = st.rearrange("p (h q) -> p h q", h=2)[:, :, col0:512]
                    att_v = att.rearrange("p (h q) -> p h q", h=2)[:, :, col0:512]
                    nc.scalar.activation(
                        att_v, st_v, mybir.ActivationFunctionType.Exp
                    )
                else:
                    nc.scalar.activation(
                        att, st, mybir.ActivationFunctionType.Exp
                    )
                if diag:
                    for r2 in range(2):
                        sl = slice(512 * r2 + col0, 512 * r2 + col0 + 128)
                        nc.gpsimd.tensor_mul(att[:, sl], att[:, sl], tri_sb)
                return att, col0

            def tile_av_part(jb, p, tsb, att, col0, avpa, avpb, first, last):
                for r2, avp in ((0, avpa), (1, avpb)):
                    nc.tensor.matmul(
                        avp[0:65, col0:512],
                        lhsT=v_t[tsb // 4][:, tsb % 4, 2 * p + r2, :],
                        rhs=att[:, 512 * r2 + col0 : 512 * (r2 + 1)],
                        start=first,
                        stop=last,
                    )

            # ---- pair finalize, two phases ----
            # phase 1 (at next pair's first tile, FIRST in the DVE queue):
            # stage y+den to SBUF (frees the AV banks) and run the 1-op
            # approximate reciprocal on the two packed den rows.
            # phase 2 (a few tiles later): f16 den, one [128,512] PSUM
            # broadcast via two column-tiled concurrent matmuls, then the
            # normalize multiplies into yt.
            def finalize1(jb, p, avpa, avpb):
                if jb == 0:
                    # all tiles of a jb=0 pair are diagonal-restricted; no
                    # full-width AV closes the accumulation group, so close
                    # it with a zero-rhs matmul (adds nothing, sets stop)
                    for avp in (avpa, avpb):
                        nc.tensor.matmul(
                            avp[0:65, :],
                            lhsT=ones_sb[:, 0:65],
                            rhs=zero16,
                            start=False,
                            stop=True,
                        )
                yra = misc.tile([65, 512], sb_dt, tag="yra", name="yra")
                yrb = misc.tile([65, 512], sb_dt, tag="yrb", name="yrb")
                nc.vector.tensor_copy(yra, avpa[0:65, :])
                nc.vector.tensor_copy(yrb, avpb[0:65, :])
                den2 = den_tiles[(4 * jb + p) % 2]
                nc.vector.tensor_copy(den2[0:1, 0:512], yra[64:65, :])
                nc.vector.tensor_copy(den2[32:33, 0:512], yrb[64:65, :])
                nc.vector.reciprocal_approx_fast(
                    den2[0:33, 512:1024], den2[0:33, 0:512]
                )
                return (jb, p, yra, yrb, den2)

            def finalize2(state):
                jb, p, yra, yrb, den2 = state
                den16 = den_pool.tile([1, 1024], sb_dt, tag="den16", name="den16")
                nc.vector.tensor_copy(den16[0:1, 0:512], den2[0:1, 512:1024])
                nc.vector.tensor_copy(den16[0:1, 512:1024], den2[32:33, 512:1024])
                bc = pa.tile([128, 512], F32, tag="apsum", name=f"bc{jb}{p}")
                nc.tensor.matmul(
                    bc[0:64, :], lhsT=ones_sb[:, 0:64], rhs=den16[0:1, 0:512],
                    tile_position=(0, 0), start=True, stop=True,
                )
                nc.tensor.matmul(
                    bc[64:128, :], lhsT=ones_sb[:, 0:64], rhs=den16[0:1, 512:1024],
                    tile_position=(0, 64), start=True, stop=True,
                )
                if p == 0:
                    yt_tiles[jb] = [
                        yt_pool.tile(
                            [128, 512], sb_dt, tag=f"yt{_o}", name=f"yt{_o}"
                        )
                        for _o in range(NOC)
                    ]
                yt = yt_tiles[jb][p]
                nc.vector.tensor_mul(yt[0:64, :], yra[0:64, :], bc[0:64, :])
                nc.vector.tensor_mul(yt[64:128, :], yrb[0:64, :], bc[64:128, :])

            # ---- tile order: diag tiles LAST so stage-A groups of the
            # NEXT jb (kt/v producers) can spill into this jb's pairs; the
            # final tile is the r=0 diag (full width) closing the group.
            def ts_order_of(jb):
                if jb == 0:
                    return [0, 3, 2, 1]
                return (
                    [0]
                    + list(range(1, 4 * jb))
                    + [4 * jb + 3, 4 * jb + 2, 4 * jb + 1, 4 * jb]
                )

            # ---- static filler plan -------------------------------------
            # filler items: (kind, key, pe_ns, deadline_slot, avail_slot)
            # slots number every attention tile globally in emission order.
            slot_of = {}
            ns = 0
            for jb in range(NJB):
                n_ts = 4 * jb + 4
                for p in range(4):
                    for idx in range(n_ts):
                        slot_of[(jb, p, idx)] = ns
                        ns += 1
            n_slots = ns

            # per-slot (act_ns, pe_ns) of the attention work itself
            slot_act = [0.0] * n_slots
            slot_pe = [0.0] * n_slots
            for jb in range(NJB):
                n_ts = 4 * jb + 4
                order = ts_order_of(jb)
                for p in range(4):
                    for idx, tsb in enumerate(order):
                        r = tsb - 4 * jb
                        col0 = 128 * r if (tsb >= 4 * jb and jb > 0) else (
                            128 * tsb if jb == 0 else 0
                        )
                        s = slot_of[(jb, p, idx)]
                        slot_act[s] = act_cost(col0)
                        slot_pe[s] = pe_att_cost(col0)

            GROUP_PE = 8 * MM_NS      # stage-A group: 8 N=512 matmuls
            CG_PE = 4 * MM_NS         # stage-C group: 4 N=512 matmuls

            fillers = []  # (avail_slot, deadline_slot, pe_ns, emit)
            for jb in range(1, NJB):
                n_ts = 4 * jb + 4
                gs = a_groups(jb)
                # order: [qk(0,q), qk(0,k), v0..v3, qk(1,q), qk(1,k), ...]
                prev_end = slot_of[(jb - 1, 0, 0)]
                dl = []
                dl.append(slot_of[(jb, 0, 0)] - 4)            # qk(0,q)
                dl.append(slot_of[(jb, 0, n_ts - 4)] - 4)     # qk(0,k)
                for tt in range(4):                           # v0..v3
                    dl.append(slot_of[(jb, 0, n_ts - 4)] - 3)
                for oc in range(1, 4):
                    dl.append(slot_of[(jb, oc, 0)] - 4)       # qk(oc,q)
                    dl.append(slot_of[(jb, oc, n_ts - 4)] - 4)  # qk(oc,k)
                for g, d in zip(gs, dl):
                    fillers.append([prev_end, d, GROUP_PE, g])
            for jb in range(NJB - 1):
                # C(jb) available once fin2(jb,3) has run (~8 tiles into
                # B(jb+1)); deadline = late in the LAST block so the
                # scheduler can bank them for the ACT-bound phases.
                avail = slot_of[(jb + 1, 0, 0)] + 9
                deadline = n_slots - 2 - (NJB - 2 - jb) * 8
                for g in c_groups(jb):
                    fillers.append([avail, deadline, CG_PE, g])
            # keep filler relative order stable (a_groups are order-
            # sensitive: the first group allocates qt)
            fq = deque(fillers)

            # ---- emission ----
            for g in a_groups(0):
                g()

            pend = None   # (jb, p, avpa, avpb) awaiting finalize1
            fin2 = None   # finalize1 state awaiting finalize2
            fin2_age = 0
            pe_clock = 12 * GROUP_PE   # a_groups(0) burst above
            act_clock = 0.0
            LEAD = 2200.0

            def pump_fillers(s):
                nonlocal pe_clock
                # force anything at deadline; else top up while PE trails ACT
                changed = True
                while changed:
                    changed = False
                    for it in list(fq):
                        av, d, cost, g = it
                        if av <= s and (d <= s or pe_clock < act_clock + LEAD):
                            fq.remove(it)
                            g()
                            pe_clock += cost
                            changed = True
                            break

            for jb in range(NJB):
                n_ts = 4 * jb + 4
                ts_order = ts_order_of(jb)
                for p in range(4):
                    avpa = avpb = None
                    avq = deque()  # tiles awaiting their AV emission
                    def flush_av(jb=jb, p=p, n_ts=n_ts):
                        nonlocal pe_clock
                        idx, tsb, att, col0 = avq.popleft()
                        tile_av_part(
                            jb, p, tsb, att, col0, avpa, avpb,
                            first=(idx == 0),
                            last=(idx == n_ts - 1 and jb > 0),
                        )
                    for idx, tsb in enumerate(ts_order):
                        s = slot_of[(jb, p, idx)]
                        att, col0 = tile_qk_part(jb, p, tsb)
                        avq.append((idx, tsb, att, col0))
                        act_clock += slot_act[s]
                        pe_clock += slot_pe[s]
                        if idx == 0:
                            if pend is not None:
                                fin2 = finalize1(*pend)
                                fin2_age = 0
                                pend = None
                            # allocate after the boundary work so the "av"
                            # ring order matches first-use order
                            avpa = pav.tile(
                                [128, 512], F32, tag="av", name=f"avpa{p}"
                            )
                            avpb = pav.tile(
                                [128, 512], F32, tag="av", name=f"avpb{p}"
                            )
                        # QK runs 2 tiles ahead of AV so the PE always has
                        # independent work while the av banks free up at
                        # pair boundaries
                        if len(avq) > 2:
                            flush_av()
                        if fin2 is not None:
                            fin2_age += 1
                            if fin2_age >= 6 or idx == n_ts - 1:
                                finalize2(fin2)
                                pe_clock += 2 * MM_NS
                                fin2 = None
                        pump_fillers(s)
                    while avq:
                        flush_av()
                    pend = (jb, p, avpa, avpb)

            # drain: finalize last pair, then the final output projection
            fin2 = finalize1(*pend)
            finalize2(fin2)
            while fq:
                fq.popleft()[3]()
            for g in c_groups(3):
                g()

    nc.finalize()
    return nc, {"np_dt": np_dt}


def shard_inputs(inputs, np_dt):
    """Full inputs -> list of 8 per-core input dicts."""
    q = np.asarray(inputs["query"], np.float32)
    Wq = np.asarray(inputs["Wq"], np.float32) * np.float32(SCALE)
    Wk = np.asarray(inputs["Wk"], np.float32)
    Wv = np.asarray(inputs["Wv"], np.float32)
    Wp = np.asarray(inputs["Wp"], np.float32)
    bq = np.asarray(inputs["bq"], np.float32)
    bk = np.asarray(inputs["bk"], np.float32)
    bv = np.asarray(inputs["bv"], np.float32)
    tri = (np.arange(128)[None, :] >= np.arange(128)[:, None]).astype(np_dt)
    in_maps = []
    for core in range(8):
        b, g = core // 2, core % 2
        sl = slice(O * g, O * (g + 1))
        in_maps.append({
            "xt": np.ascontiguousarray(
                q[b].T.reshape(8, 128, NJB, 512).transpose(2, 1, 0, 3)
            ).astype(np_dt),
            "wqt": np.ascontiguousarray(
                Wq[sl, :].T.reshape(8, 128, O).transpose(1, 0, 2)
            ).astype(np_dt),
            "wkt": np.ascontiguousarray(
                Wk[sl, :].T.reshape(8, 128, O).transpose(1, 0, 2)
            ).astype(np_dt),
            "wvt": np.ascontiguousarray(
                Wv[sl, :].T.reshape(8, 128, O).transpose(1, 0, 2)
            ).astype(np_dt),
            "wpt": np.ascontiguousarray(
                Wp[:, sl].T.reshape(NOC, 128, C).transpose(1, 0, 2)
            ).astype(np_dt),
            "bq": np.ascontiguousarray(bq[sl].reshape(NOC, 128).T) * np.float32(SCALE),
            "bk": np.ascontiguousarray(bk[sl].reshape(NOC, 128).T),
            "bvb": np.broadcast_to(bv[sl], (128, O)).copy(),
            "tri": tri,
        })
    return in_maps


def unshard(results, bp):
    out = np.empty((4, T, C), np.float32)
    bp32 = np.asarray(bp, np.float32)
    for b in range(4):
        out[b] = (
            results[2 * b]["out"].astype(np.float32)
            + results[2 * b + 1]["out"].astype(np.float32)
            + bp32
        )
    return out


_CACHE = {}


def _get_nc(mode="f16"):
    if mode not in _CACHE:
        _CACHE[mode] = build()
    return _CACHE[mode]


def kernel(**inputs):
    """Full unsharded inputs -> full [4, 2048, 1024] fp32 output."""
    from concourse import bass_utils

    nc, meta = _get_nc("f16")
    in_maps = shard_inputs(inputs, meta["np_dt"])
    res = bass_utils.run_bass_kernel_spmd(nc, in_maps, list(range(8)))
    return unshard(res.results, inputs["bp"])


# revision 10
# speedup vs baseline: 1.1666x; 1.0209x over previous
"""Self-contained Trainium2 Bass kernel: causal self-attention, 8-core SPMD.

nn_CausalSelfAttention: B=4, T=2048, C=1024, n_head=16 (fp32 reference).

Sharding (hardcoded): core c -> batch b = c//2, head-group g = c%2
(8 of 16 heads = 512 features). Data parallel over B, tensor parallel
over heads. Each core computes a partial output [T, C] = y_g @ Wp_g^T;
the host sums the two partials per batch and adds bp (the tensor-parallel
all-reduce done at unshard time).

v3 changes over the 353us v2 baseline (trace-driven):
  - pair-finalize reciprocal switched to the 1-op approx DVE reciprocal
    (3.34us InstReciprocal -> ~0.7us) so the DVE in-order queue no
    longer backs up behind it at pair boundaries (the v2 trace showed
    2.4-2.8us PE gaps at every boundary, each re-throttling the PE
    clock to 1.2 GHz for ~10us via HAM)
  - q/k bias evictions moved from ScalarE (IDENTITY, 22us) to DVE
    tensor_scalar so ScalarE runs exp only; exp throughput is the
    secondary bottleneck in the late (large-jb) phases
  - output-projection groups moved off the "av" PSUM ring onto the
    "pa" ring so they never wait on pair-finalize copies
  - den broadcast packed into one [128,512] PSUM tile via two
    column-tiled concurrent matmuls (tile_position (0,0)/(0,64))
  - diag tiles processed LAST within each pair so stage-A groups of
    block jb+1 can spill into B(jb+1) itself; filler emission is
    driven by a static PE-vs-ACT cost model instead of fixed strides
  - initial DMAs split across 4 queue engines (v2 spent 19us before
    the first matmul); fp16 partial outputs (halves the output DMA)
~6e-4 relative error vs the fp32 reference.
"""

import sys
from collections import deque

for _p in ("/opt/trn_rl_repo",):
    if _p not in sys.path:
        sys.path.insert(0, _p)

import numpy as np

import concourse.bacc as bacc
import concourse.bass as bass
import concourse.tile as tile
from concourse import mybir

F32 = mybir.dt.float32
F16 = mybir.dt.float16

T = 2048
C = 1024
O = 512          # per-core output features (8 heads x 64)
HD = 64
NJB = 4          # tq blocks of 512
NCC = 8          # c chunks of 128
NOC = 4          # o chunks of 128
SCALE = 1.0 / 8.0  # 1/sqrt(64), folded into Wq/bq host-side

# cost model (ns) for the static filler scheduler
MM_NS = 216.0        # N=512 matmul, warm
EXP_OVH = 352 / 1.2  # ACT per-instruction overhead


def act_cost(col0):
    """exp cost for one tile (2 heads), diag-restricted to [col0:512]."""
    return (2 * (512 - col0) + 352) / 1.2


def pe_att_cost(col0):
    """QK (row-tiled concurrent pair) + 2 AV matmuls for one tile."""
    return (3 * (512 - col0)) / 2.4 + 24.0


def build(debug=False):
    np_dt = np.float16
    sb_dt = F16

    nc = bacc.Bacc("TRN2", target_bir_lowering=False, debug=False)

    # all large inputs come as SBUF images ([128 partitions, ...]) so each
    # DMA reads 128 contiguous multi-KB rows instead of 1024 strided 1KB rows
    xt_d = nc.dram_tensor("xt", [NJB, 128, NCC, 512], sb_dt, kind="ExternalInput").ap()
    wqt_d = nc.dram_tensor("wqt", [128, NCC, O], sb_dt, kind="ExternalInput").ap()
    wkt_d = nc.dram_tensor("wkt", [128, NCC, O], sb_dt, kind="ExternalInput").ap()
    wvt_d = nc.dram_tensor("wvt", [128, NCC, O], sb_dt, kind="ExternalInput").ap()
    wpt_d = nc.dram_tensor("wpt", [128, NOC, C], sb_dt, kind="ExternalInput").ap()
    bq_d = nc.dram_tensor("bq", [128, NOC], F32, kind="ExternalInput").ap()
    bk_d = nc.dram_tensor("bk", [128, NOC], F32, kind="ExternalInput").ap()
    bvb_d = nc.dram_tensor("bvb", [128, O], F32, kind="ExternalInput").ap()
    tri_d = nc.dram_tensor("tri", [128, 128], sb_dt, kind="ExternalInput").ap()
    out_d = nc.dram_tensor("out", [T, C], sb_dt, kind="ExternalOutput").ap()

    with tile.TileContext(nc) as tc:
        with (
            tc.tile_pool(name="const", bufs=1) as const,
            tc.tile_pool(name="qt_pool", bufs=2) as qt_pool,
            tc.tile_pool(name="att_pool", bufs=4) as att_pool,
            tc.tile_pool(name="yt_pool", bufs=16) as yt_pool,
            tc.tile_pool(name="den_pool", bufs=2) as den_pool,
            tc.tile_pool(name="misc", bufs=4) as misc,
            tc.tile_pool(name="ost_pool", bufs=3) as ost_pool,
            tc.tile_pool(name="pst", bufs=2, space="PSUM") as pst,
            tc.tile_pool(name="pa", bufs=2, space="PSUM") as pa,
            tc.tile_pool(name="pav", bufs=2, space="PSUM") as pav,
        ):
            # ---- constants / weights ----
            # small tensors first (biases/tri are needed by the first
            # evictions), then the first-group-critical big tensors split
            # across the 4 queue engines so the first matmul can start
            # ~7us in instead of 19us.
            wq_sb = const.tile([128, NCC, O], sb_dt, name="wq_sb")
            wk_sb = const.tile([128, NCC, O], sb_dt, name="wk_sb")
            wv_sb = const.tile([128, NCC, O], sb_dt, name="wv_sb")
            wp_sb = const.tile([128, NOC, C], sb_dt, name="wp_sb")
            xt_sb = const.tile([128, NJB, NCC, 512], sb_dt, name="xt_sb")
            bq_sb = const.tile([128, NOC], F32, name="bq_sb")
            bk_sb = const.tile([128, NOC], F32, name="bk_sb")
            bvb_sb = const.tile([128, O], F32, name="bvb_sb")
            tri_sb = const.tile([128, 128], sb_dt, name="tri_sb")

            # strict consumption order, finely split round-robin over the
            # 3 queue engines: the A(0) burst consumes wq-oc0 + xt0 first,
            # then wk-oc0 (kt for the first attention tile), then wv (any
            # v group streams all 512 wv columns), then the rest.
            qs = [nc.sync, nc.scalar, nc.gpsimd]
            qi = 0

            def dma(out, in_):
                nonlocal qi
                qs[qi % 3].dma_start(out=out, in_=in_)
                qi += 1

            dma(bq_sb, bq_d)
            for cc in range(4):
                dma(wq_sb[:, 2 * cc : 2 * cc + 2, 0:128],
                    wqt_d[:, 2 * cc : 2 * cc + 2, 0:128])
                dma(xt_sb[:, 0, 2 * cc : 2 * cc + 2],
                    xt_d[0, :, 2 * cc : 2 * cc + 2])
            for cc in range(2):
                dma(wk_sb[:, 4 * cc : 4 * cc + 4, 0:128],
                    wkt_d[:, 4 * cc : 4 * cc + 4, 0:128])
            dma(tri_sb, tri_d)
            dma(bvb_sb, bvb_d)
            for i in range(4):
                dma(wv_sb[:, :, 128 * i : 128 * (i + 1)],
                    wvt_d[:, :, 128 * i : 128 * (i + 1)])
            dma(bk_sb, bk_d)
            for i in range(1, 4):
                dma(wq_sb[:, :, 128 * i : 128 * (i + 1)],
                    wqt_d[:, :, 128 * i : 128 * (i + 1)])
                dma(wk_sb[:, :, 128 * i : 128 * (i + 1)],
                    wkt_d[:, :, 128 * i : 128 * (i + 1)])
            dma(xt_sb[:, 1], xt_d[1])
            dma(xt_sb[:, 2], xt_d[2])
            dma(xt_sb[:, 3], xt_d[3])
            dma(wp_sb, wpt_d)

            # denominator staging: two const tiles used alternately by the
            # pair finalizes (rows 1-31 hold the 1.0 memset so the packed
            # approximate reciprocal never sees garbage; partition bases
            # must be 32-aligned, hence rows 0 and 32 for the two heads)
            den_tiles = []
            for dd in range(2):
                dt_ = const.tile([33, 1024], F32, name=f"den{dd}")
                nc.vector.memset(dt_[:, 0:512], 1.0)
                den_tiles.append(dt_)

            # ones row for the K=1 den broadcast + zero row for the jb=0
            # group-closing matmul
            ones_sb = const.tile([1, 128], sb_dt, name="ones_sb")
            nc.vector.memset(ones_sb, 1.0)
            zero16 = const.tile([1, 512], sb_dt, name="zero16")
            nc.vector.memset(zero16, 0.0)

            # persistent K^T and V per (chunk, block). V carries a ones
            # column per head ([v | 1]) so the AV matmul (M=65) also
            # accumulates the softmax denominator in its row 64.
            kt_t = {}
            v_t = {}
            for jbx in range(NJB):
                for oc in range(NOC):
                    kt_t[oc, jbx] = const.tile(
                        [128, 512], sb_dt, name=f"kt{oc}_{jbx}"
                    )
                v_t[jbx] = const.tile([128, 4, 8, 65], sb_dt, name=f"v_{jbx}")
                nc.vector.memset(v_t[jbx][:, :, :, 64:65], 1.0)

            qt_tiles = {}
            yt_tiles = {}

            # ---- stage A: QKV projection groups for t-block jb ----
            def qk_group(jb, oc, mat):
                def emit():
                    if mat == 0 and oc == 0:
                        qt_tiles[jb] = qt_pool.tile(
                            [128, NOC, 512], sb_dt, tag="qt", name="qt"
                        )
                    w_sb = wq_sb if mat == 0 else wk_sb
                    ps = pa.tile([128, 512], F32, tag="apsum", name=f"qk{jb}{oc}{mat}")
                    for cc in range(NCC):
                        nc.tensor.matmul(
                            ps,
                            lhsT=w_sb[:, cc, 128 * oc : 128 * (oc + 1)],
                            rhs=xt_sb[:, jb, cc, :],
                            start=(cc == 0),
                            stop=(cc == NCC - 1),
                        )
                    if mat == 0:
                        nc.vector.tensor_scalar_add(
                            qt_tiles[jb][:, oc, :], ps, bq_sb[:, oc : oc + 1]
                        )
                    else:
                        nc.vector.tensor_scalar_add(
                            kt_t[oc, jb], ps, bk_sb[:, oc : oc + 1]
                        )
                return emit

            def v_group(jb, tt):
                def emit():
                    ps = pa.tile([128, 512], F32, tag="apsum", name=f"v{jb}{tt}")
                    for cc in range(NCC):
                        nc.tensor.matmul(
                            ps,
                            lhsT=xt_sb[:, jb, cc, 128 * tt : 128 * (tt + 1)],
                            rhs=wv_sb[:, cc, :],
                            start=(cc == 0),
                            stop=(cc == NCC - 1),
                        )
                    nc.vector.scalar_tensor_tensor(
                        v_t[jb][:, tt, :, 0:64],
                        ps.rearrange("p (h d) -> p h d", h=8),
                        0.0,
                        bvb_sb.rearrange("p (h d) -> p h d", h=8),
                        op0=mybir.AluOpType.add,
                        op1=mybir.AluOpType.add,
                    )
                return emit

            def a_groups(jb):
                gs = [qk_group(jb, 0, 0), qk_group(jb, 0, 1)]
                gs += [v_group(jb, tt) for tt in range(4)]
                for oc in range(1, 4):
                    gs += [qk_group(jb, oc, 0), qk_group(jb, oc, 1)]
                return gs

            # ---- stage C: output projection group for t-block jb ----
            def c_group(jb, cb, tt):
                def emit():
                    op = pa.tile([128, 512], F32, tag="apsum", name=f"op{jb}{cb}{tt}")
                    yt_c = yt_tiles[jb]
                    for oc in range(NOC):
                        nc.tensor.matmul(
                            op,
                            lhsT=yt_c[oc][:, 128 * tt : 128 * (tt + 1)],
                            rhs=wp_sb[:, oc, 512 * cb : 512 * (cb + 1)],
                            start=(oc == 0),
                            stop=(oc == NOC - 1),
                        )
                    ost = ost_pool.tile([128, 512], sb_dt, tag="ost", name="ost")
                    nc.vector.tensor_copy(ost, op)
                    dq = qs[(cb * 4 + tt) % 3]
                    dq.dma_start(
                        out=out_d[
                            512 * jb + 128 * tt : 512 * jb + 128 * (tt + 1),
                            512 * cb : 512 * (cb + 1),
                        ],
                        in_=ost,
                    )
                return emit

            def c_groups(jb):
                return [c_group(jb, cb, tt) for cb in range(2) for tt in range(4)]

            # ---- stage B tile: QK^T scores -> exp -> (mask) -> AV ----
            def tile_qk_part(jb, p, tsb):
                diag = tsb >= 4 * jb
                r = tsb - 4 * jb
                col0 = 128 * r if diag else 0
                st = pst.tile([128, 1024], F32, tag="st", name="st")
                for r2 in range(2):
                    nc.tensor.matmul(
                        st[:, 512 * r2 + col0 : 512 * (r2 + 1)],
                        lhsT=kt_t[p, tsb // 4][
                            64 * r2 : 64 * (r2 + 1),
                            128 * (tsb % 4) : 128 * (tsb % 4 + 1),
                        ],
                        rhs=qt_tiles[jb][64 * r2 : 64 * (r2 + 1), p, col0:512],
                        tile_position=(64 * r2, 0),
                        start=True,
                        stop=True,
                    )
                att = att_pool.tile([128, 1024], sb_dt, tag="att", name="att")
                if col0:
                    st_v = st.rearrange("p (h q) -> p h q", h=2)[:, :, col0:512]
                    att_v = att.rearrange("p (h q) -> p h q", h=2)[:, :, col0:512]
                    nc.scalar.activation(
                        att_v, st_v, mybir.ActivationFunctionType.Exp
                    )
                else:
                    nc.scalar.activation(
                        att, st, mybir.ActivationFunctionType.Exp
                    )
                if diag:
                    for r2 in range(2):
                        sl = slice(512 * r2 + col0, 512 * r2 + col0 + 128)
                        nc.gpsimd.tensor_mul(att[:, sl], att[:, sl], tri_sb)
                return att, col0

            def tile_av_part(jb, p, tsb, att, col0, avpa, avpb, first, last):
                for r2, avp in ((0, avpa), (1, avpb)):
                    nc.tensor.matmul(
                        avp[0:65, col0:512],
                        lhsT=v_t[tsb // 4][:, tsb % 4, 2 * p + r2, :],
                        rhs=att[:, 512 * r2 + col0 : 512 * (r2 + 1)],
                        start=first,
                        stop=last,
                    )

            # ---- pair finalize, two phases ----
            # phase 1 (at next pair's first tile, FIRST in the DVE queue):
            # stage y+den to SBUF (frees the AV banks) and run the 1-op
            # approximate reciprocal on the two packed den rows.
            # phase 2 (a few tiles later): f16 den, one [128,512] PSUM
            # broadcast via two column-tiled concurrent matmuls, then the
            # normalize multiplies into yt.
            def finalize1(jb, p, avpa, avpb):
                if jb == 0:
                    # all tiles of a jb=0 pair are diagonal-restricted; no
                    # full-width AV closes the accumulation group, so close
                    # it with a zero-rhs matmul (adds nothing, sets stop)
                    for avp in (avpa, avpb):
                        nc.tensor.matmul(
                            avp[0:65, :],
                            lhsT=ones_sb[:, 0:65],
                            rhs=zero16,
                            start=False,
                            stop=True,
                        )
                yra = misc.tile([65, 512], sb_dt, tag="yra", name="yra")
                yrb = misc.tile([65, 512], sb_dt, tag="yrb", name="yrb")
                nc.vector.tensor_copy(yra, avpa[0:65, :])
                nc.vector.tensor_copy(yrb, avpb[0:65, :])
                den2 = den_tiles[(4 * jb + p) % 2]
                nc.vector.tensor_copy(den2[0:1, 0:512], yra[64:65, :])
                nc.vector.tensor_copy(den2[32:33, 0:512], yrb[64:65, :])
                nc.vector.reciprocal_approx_fast(
                    den2[0:33, 512:1024], den2[0:33, 0:512]
                )
                return (jb, p, yra, yrb, den2)

            def finalize2(state):
                jb, p, yra, yrb, den2 = state
                den16 = den_pool.tile([1, 1024], sb_dt, tag="den16", name="den16")
                nc.vector.tensor_copy(den16[0:1, 0:512], den2[0:1, 512:1024])
                nc.vector.tensor_copy(den16[0:1, 512:1024], den2[32:33, 512:1024])
                bc = pa.tile([128, 512], F32, tag="apsum", name=f"bc{jb}{p}")
                nc.tensor.matmul(
                    bc[0:64, :], lhsT=ones_sb[:, 0:64], rhs=den16[0:1, 0:512],
                    tile_position=(0, 0), start=True, stop=True,
                )
                nc.tensor.matmul(
                    bc[64:128, :], lhsT=ones_sb[:, 0:64], rhs=den16[0:1, 512:1024],
                    tile_position=(0, 64), start=True, stop=True,
                )
                if p == 0:
                    yt_tiles[jb] = [
                        yt_pool.tile(
                            [128, 512], sb_dt, tag=f"yt{_o}", name=f"yt{_o}"
                        )
                        for _o in range(NOC)
                    ]
                yt = yt_tiles[jb][p]
                nc.vector.tensor_mul(yt[0:64, :], yra[0:64, :], bc[0:64, :])
                nc.vector.tensor_mul(yt[64:128, :], yrb[0:64, :], bc[64:128, :])

            # ---- tile order: diag tiles LAST so stage-A groups of the
            # NEXT jb (kt/v producers) can spill into this jb's pairs; the
            # final tile is the r=0 diag (full width) closing the group.
            def ts_order_of(jb):
                if jb == 0:
                    return [0, 3, 2, 1]
                return (
                    [0]
                    + list(range(1, 4 * jb))
                    + [4 * jb + 3, 4 * jb + 2, 4 * jb + 1, 4 * jb]
                )

            # ---- static filler plan -------------------------------------
            # filler items: (kind, key, pe_ns, deadline_slot, avail_slot)
            # slots number every attention tile globally in emission order.
            slot_of = {}
            ns = 0
            for jb in range(NJB):
                n_ts = 4 * jb + 4
                for p in range(4):
                    for idx in range(n_ts):
                        slot_of[(jb, p, idx)] = ns
                        ns += 1
            n_slots = ns

            # per-slot (act_ns, pe_ns) of the attention work itself
            slot_act = [0.0] * n_slots
            slot_pe = [0.0] * n_slots
            for jb in range(NJB):
                n_ts = 4 * jb + 4
                order = ts_order_of(jb)
                for p in range(4):
                    for idx, tsb in enumerate(order):
                        r = tsb - 4 * jb
                        col0 = 128 * r if (tsb >= 4 * jb and jb > 0) else (
                            128 * tsb if jb == 0 else 0
                        )
                        s = slot_of[(jb, p, idx)]
                        slot_act[s] = act_cost(col0)
                        slot_pe[s] = pe_att_cost(col0)

            GROUP_PE = 8 * MM_NS      # stage-A group: 8 N=512 matmuls
            CG_PE = 4 * MM_NS         # stage-C group: 4 N=512 matmuls

            fillers = []  # (avail_slot, deadline_slot, pe_ns, emit)
            for jb in range(1, NJB):
                n_ts = 4 * jb + 4
                gs = a_groups(jb)
                # order: [qk(0,q), qk(0,k), v0..v3, qk(1,q), qk(1,k), ...]
                prev_end = slot_of[(jb - 1, 0, 0)]
                dl = []
                dl.append(slot_of[(jb, 0, 0)] - 4)            # qk(0,q)
                dl.append(slot_of[(jb, 0, n_ts - 4)] - 4)     # qk(0,k)
                for tt in range(4):                           # v0..v3
                    dl.append(slot_of[(jb, 0, n_ts - 4)] - 3)
                for oc in range(1, 4):
                    dl.append(slot_of[(jb, oc, 0)] - 4)       # qk(oc,q)
                    dl.append(slot_of[(jb, oc, n_ts - 4)] - 4)  # qk(oc,k)
                for g, d in zip(gs, dl):
                    fillers.append([prev_end, d, GROUP_PE, g])
            for jb in range(NJB - 1):
                # C(jb) available once fin2(jb,3) has run (~8 tiles into
                # B(jb+1)); deadline = late in the LAST block so the
                # scheduler can bank them for the ACT-bound phases.
                avail = slot_of[(jb + 1, 0, 0)] + 9
                deadline = n_slots - 2 - (NJB - 2 - jb) * 8
                for g in c_groups(jb):
                    fillers.append([avail, deadline, CG_PE, g])
            # keep filler relative order stable (a_groups are order-
            # sensitive: the first group allocates qt)
            fq = deque(fillers)

            # ---- emission ----
            for g in a_groups(0):
                g()

            pend = None   # (jb, p, avpa, avpb) awaiting finalize1
            fin2 = None   # finalize1 state awaiting finalize2
            fin2_age = 0
            pe_clock = 12 * GROUP_PE   # a_groups(0) burst above
            act_clock = 0.0
            LEAD = 1500.0

            def pump_fillers(s):
                nonlocal pe_clock
                # force anything at deadline; else top up (max 2 per slot
                # so the filler supply is rationed across ACT-bound
                # stretches instead of being burned in a burst) while PE
                # trails ACT
                pulled = 0
                changed = True
                while changed:
                    changed = False
                    for it in list(fq):
                        av, d, cost, g = it
                        due = d <= s
                        if av <= s and (
                            due or (pulled < 2 and pe_clock < act_clock + LEAD)
                        ):
                            fq.remove(it)
                            g()
                            pe_clock += cost
                            if not due:
                                pulled += 1
                            changed = True
                            break

            for jb in range(NJB):
                n_ts = 4 * jb + 4
                ts_order = ts_order_of(jb)
                for p in range(4):
                    avpa = avpb = None
                    avq = deque()  # tiles awaiting their AV emission
                    def flush_av(jb=jb, p=p, n_ts=n_ts):
                        nonlocal pe_clock
                        idx, tsb, att, col0 = avq.popleft()
                        tile_av_part(
                            jb, p, tsb, att, col0, avpa, avpb,
                            first=(idx == 0),
                            last=(idx == n_ts - 1 and jb > 0),
                        )
                    for idx, tsb in enumerate(ts_order):
                        s = slot_of[(jb, p, idx)]
                        att, col0 = tile_qk_part(jb, p, tsb)
                        avq.append((idx, tsb, att, col0))
                        act_clock += slot_act[s]
                        pe_clock += slot_pe[s]
                        if idx == 0:
                            if pend is not None:
                                fin2 = finalize1(*pend)
                                fin2_age = 0
                                pend = None
                            # allocate after the boundary work so the "av"
                            # ring order matches first-use order
                            avpa = pav.tile(
                                [128, 512], F32, tag="av", name=f"avpa{p}"
                            )
                            avpb = pav.tile(
                                [128, 512], F32, tag="av", name=f"avpb{p}"
                            )
                        # QK runs 2 tiles ahead of AV so the PE always has
                        # independent work while the av banks free up at
                        # pair boundaries
                        if len(avq) > 2:
                            flush_av()
                        if fin2 is not None:
                            fin2_age += 1
                            if fin2_age >= 6 or idx == n_ts - 1:
                                finalize2(fin2)
                                pe_clock += 2 * MM_NS
                                fin2 = None
                        pump_fillers(s)
                    while avq:
                        flush_av()
                    pend = (jb, p, avpa, avpb)

            # drain: finalize last pair, then the final output projection
            fin2 = finalize1(*pend)
            finalize2(fin2)
            while fq:
                fq.popleft()[3]()
            for g in c_groups(3):
                g()

    nc.finalize()
    return nc, {"np_dt": np_dt}


def shard_inputs(inputs, np_dt):
    """Full inputs -> list of 8 per-core input dicts."""
    q = np.asarray(inputs["query"], np.float32)
    Wq = np.asarray(inputs["Wq"], np.float32) * np.float32(SCALE)
    Wk = np.asarray(inputs["Wk"], np.float32)
    Wv = np.asarray(inputs["Wv"], np.float32)
    Wp = np.asarray(inputs["Wp"], np.float32)
    bq = np.asarray(inputs["bq"], np.float32)
    bk = np.asarray(inputs["bk"], np.float32)
    bv = np.asarray(inputs["bv"], np.float32)
    tri = (np.arange(128)[None, :] >= np.arange(128)[:, None]).astype(np_dt)
    in_maps = []
    for core in range(8):
        b, g = core // 2, core % 2
        sl = slice(O * g, O * (g + 1))
        in_maps.append({
            "xt": np.ascontiguousarray(
                q[b].T.reshape(8, 128, NJB, 512).transpose(2, 1, 0, 3)
            ).astype(np_dt),
            "wqt": np.ascontiguousarray(
                Wq[sl, :].T.reshape(8, 128, O).transpose(1, 0, 2)
            ).astype(np_dt),
            "wkt": np.ascontiguousarray(
                Wk[sl, :].T.reshape(8, 128, O).transpose(1, 0, 2)
            ).astype(np_dt),
            "wvt": np.ascontiguousarray(
                Wv[sl, :].T.reshape(8, 128, O).transpose(1, 0, 2)
            ).astype(np_dt),
            "wpt": np.ascontiguousarray(
                Wp[:, sl].T.reshape(NOC, 128, C).transpose(1, 0, 2)
            ).astype(np_dt),
            "bq": np.ascontiguousarray(bq[sl].reshape(NOC, 128).T) * np.float32(SCALE),
            "bk": np.ascontiguousarray(bk[sl].reshape(NOC, 128).T),
            "bvb": np.broadcast_to(bv[sl], (128, O)).copy(),
            "tri": tri,
        })
    return in_maps


def unshard(results, bp):
    out = np.empty((4, T, C), np.float32)
    bp32 = np.asarray(bp, np.float32)
    for b in range(4):
        out[b] = (
            results[2 * b]["out"].astype(np.float32)
            + results[2 * b + 1]["out"].astype(np.float32)
            + bp32
        )
    return out


_CACHE = {}


def _get_nc(mode="f16"):
    if mode not in _CACHE:
        _CACHE[mode] = build()
    return _CACHE[mode]


def kernel(**inputs):
    """Full unsharded inputs -> full [4, 2048, 1024] fp32 output."""
    from concourse import bass_utils

    nc, meta = _get_nc("f16")
    in_maps = shard_inputs(inputs, meta["np_dt"])
    res = bass_utils.run_bass_kernel_spmd(nc, in_maps, list(range(8)))
    return unshard(res.results, inputs["bp"])
